# revision 1
# baseline (speedup 1.0000x reference)
"""InfoNCE lower-bound kernel for 8 Trainium2 NeuronCores (v2).

Math (reference):
  hx = x @ W1x.T ; hy = y @ W1y.T            [N, H]
  z_ij = relu(hx[j] + hy[i] + b1) . w2       (logit WITHOUT b2)
  T1[i,j] = softplus(z_ij + b2)
  T0[i]   = T1[i,i]
  lse[i]  = log(sum_j exp(T1[i,j])) = log(N + sum_j exp(z_ij + b2))
  out     = mean(T0) - (mean(lse) - log N)

Sharding: data-parallel over i (rows of the pair grid); each core gets 64
rows of y, x and params replicated.

v2 design decisions (vs the fp32 baseline):
  * bf16 operands everywhere on the grid path: matmuls run at 1 cycle/row
    (fp32 pays 4), DVE elementwise runs in packed mode, DMA bytes halve.
  * The relu grid pass is split across DVE and Activation via a tunable
    assignment table (Pool's tensor_scalar ucode is ~8us/op — unusable).
  * H=300 is tiled 128+128+44; the 44-tail relu work for two adjacent rows
    is packed into one [128,512] op (rows 0:44 and 64:108), cutting the
    elementwise op count from 192 to 160.
  * z matvecs use an M=16 stationary [K,16] holding w2 in column q_i
    (zeros elsewhere), at tile_position col strips (0,32c): row i lands at
    PSUM partition 32c+q of bank c (c=(i//2)%4, q=2(i//8)+i%2). Each strip
    accumulates into its OWN bank: concurrent col-strip matmuls accumulating
    into one bank corrupt each other on HW (verified in isolation). All
    matvecs are K=128 at row position 0 (a packed t2 pair is ONE matmul
    with w2 in col q0 rows 0:44 and col q0+1 rows 64:108); adding the zero
    columns is exact in fp32 PSUM.
  * exp then reads each bank directly with full-128-partition ops
    (partition-sliced Act reads of PSUM only process one partition on HW),
    followed by one DVE free-axis reduce per bank. No extraction copies.
  * Device ships per-row partial results (sum_j exp(z+b2) and diag logits);
    the final ln/softplus/means over 128 floats per core run on the host.
"""

import math

import numpy as np

N = 512
XD = 768
YD = 768
H = 300
NCORES = 8
ISH = N // NCORES   # 64 rows per core
KD = XD // 128      # 6 contraction tiles of 128
HT = 3              # h tiles: 128, 128, 44
HSZ = [128, 128, H - 256]

# Engine assignment for the 20 relu ops per block of 8 rows, in emission
# order [8x t0, 8x t1, 4x t2-pair]. 'D' = DVE (vector), 'A' = Activation
# (scalar). Pool (gpsimd) is useless here: its tensor_scalar ucode measured
# ~8 us per [128,512] op.
PATTERN20_6A = (['D', 'A', 'D', 'D', 'D', 'D', 'A', 'D'] * 2 + ['A', 'A', 'D', 'D'])
PATTERN20_5A = (['D', 'A', 'D', 'D', 'D', 'D', 'A', 'D'] * 2 + ['A', 'D', 'D', 'D'])

_CACHE = {}
TRACE = False
LAST_RESULTS = None


def _build_module():
    import concourse.bacc as bacc
    import concourse.mybir as mybir
    from concourse.tile import TileContext

    f32 = mybir.dt.float32
    bf16 = mybir.dt.bfloat16
    AF = mybir.ActivationFunctionType
    ALU = mybir.AluOpType

    nc = bacc.Bacc("TRN2", target_bir_lowering=False, debug=False)

    # Per-core inputs (SPMD: same shapes, different data for yT/xTd).
    xT = nc.dram_tensor("xT", [XD, N], bf16, kind="ExternalInput")      # x^T
    w1xT = nc.dram_tensor("w1xT", [XD, H], bf16, kind="ExternalInput")  # W1x^T
    w1yT = nc.dram_tensor("w1yT", [YD, H], bf16, kind="ExternalInput")  # W1y^T
    yT = nc.dram_tensor("yT", [YD, ISH], bf16, kind="ExternalInput")    # y-slice^T
    xTd = nc.dram_tensor("xTd", [XD, ISH], bf16, kind="ExternalInput")  # x-slice^T
    bcons = nc.dram_tensor("bcons", [128, HT + 1], f32, kind="ExternalInput")  # b1 cols 0:3, b2 col 3
    w2all = nc.dram_tensor("w2all", [128, 4 + 40 * 16], bf16, kind="ExternalInput")  # diag w2 | w2q blocks
    b1row = nc.dram_tensor("b1row", [1, 3 * 128], bf16, kind="ExternalInput")  # b1 chunks as rows
    outS = nc.dram_tensor("outS", [ISH, 1], f32, kind="ExternalOutput")  # sum_j exp(z+b2)
    outD = nc.dram_tensor("outD", [1, ISH], f32, kind="ExternalOutput")  # diag logits

    with TileContext(nc) as tc:
        cpool = tc.alloc_tile_pool(name="consts", bufs=1)
        rpool = tc.alloc_tile_pool(name="work", bufs=32)
        pp_pre = tc.alloc_tile_pool(name="pp_pre", bufs=1, space="PSUM")
        pp_z = tc.alloc_tile_pool(name="pp_z", bufs=1, space="PSUM")
        pp_d = tc.alloc_tile_pool(name="pp_d", bufs=1, space="PSUM")

        # ---- constant tiles ----
        xt_sb = cpool.tile([128, KD * N], bf16, tag="xt")
        w1x_sb = cpool.tile([128, KD * H], bf16, tag="w1x")
        w1y_sb = cpool.tile([128, KD * H], bf16, tag="w1y")
        yt_sb = cpool.tile([128, KD * ISH], bf16, tag="yt")
        xtd_sb = cpool.tile([128, KD * ISH], bf16, tag="xtd")
        bc_sb = cpool.tile([128, HT + 1], f32, tag="bc")
        w2a_sb = cpool.tile([128, 4 + 40 * 16], bf16, tag="w2a")

        hxb0 = cpool.tile([128, N], bf16, tag="hxb0")    # relu-arg x part (+b1), t0
        hxb1 = cpool.tile([128, N], bf16, tag="hxb1")    # t1
        hxb2p = cpool.tile([128, N], bf16, tag="hxb2p")  # t2 packed (rows 0:44, 64:108)
        hy_sb = cpool.tile([128, 2 * ISH], bf16, tag="hy")     # t0 | t1 columns
        hyf_sb = cpool.tile([128, HT * ISH], f32, tag="hyf")   # fp32 twin (scalar/bias/diag)
        hy2p = cpool.tile([128, ISH // 2], f32, tag="hy2p")    # packed t2 pairs
        b1r_sb = cpool.tile([1, 3 * 128], bf16, tag="b1r")
        ones64 = cpool.tile([1, ISH], bf16, tag="ones64")
        ee = cpool.tile([128, 4 * N], bf16, tag="ee")
        sexp4 = cpool.tile([128, 4], f32, tag="sexp4")
        dlog = cpool.tile([1, ISH], f32, tag="dlog")

        # ---- input DMAs: one queue, critical path (hxb0 = xT+w1x) first,
        # split into k-chunk halves so the psum accumulation can start on the
        # first half while the second lands.
        KH = KD // 2
        def half(dst, srcT, h, kd=KD):
            lo, hi = h * (kd // 2), (h + 1) * (kd // 2)
            nc.sync.dma_start(
                dst[:, lo * (dst.shape[1] // kd):hi * (dst.shape[1] // kd)]
                .rearrange("p (k f) -> p k f", k=kd // 2),
                srcT[lo * 128:hi * 128, :].rearrange("(k p) f -> p k f", k=kd // 2),
            )
        half(xt_sb, xT, 0)
        half(w1x_sb, w1xT, 0)
        half(w1y_sb, w1yT, 0)
        nc.sync.dma_start(
            yt_sb[:].rearrange("p (k f) -> p k f", k=KD),
            yT[:, :].rearrange("(k p) f -> p k f", k=KD),
        )
        half(xt_sb, xT, 1)
        half(w1x_sb, w1xT, 1)
        half(w1y_sb, w1yT, 1)
        nc.sync.dma_start(bc_sb[:], bcons[:])
        nc.sync.dma_start(w2a_sb[:], w2all[:])
        nc.sync.dma_start(
            xtd_sb[:].rearrange("p (k f) -> p k f", k=KD),
            xTd[:, :].rearrange("(k p) f -> p k f", k=KD),
        )
        nc.sync.dma_start(b1r_sb[:], b1row[:])

        nc.gpsimd.memset(ones64[:], 1.0)
        # zero the packed-t2 operand tiles before their writers fill the
        # live rows, so the pair matmul's zero-weight rows multiply finite
        # values (NaN * 0 = NaN).
        nc.gpsimd.memset(hxb2p[:], 0.0)
        nc.gpsimd.memset(hy2p[:], 0.0)

        # ---- preamble: hxb (x part, +b1) and hy interleaved per h-tile ----
        for t in range(HT):
            hs = HSZ[t]
            ps = pp_pre.tile([128, N], f32, tag="pre512", bufs=2)
            for k in range(KD):
                nc.tensor.matmul(
                    ps[0:hs, :],
                    lhsT=w1x_sb[:, k * H + 128 * t: k * H + 128 * t + hs],
                    rhs=xt_sb[:, k * N:(k + 1) * N],
                    start=(k == 0), stop=(k == KD - 1),
                )
            dst = [hxb0, hxb1, hxb2p][t]
            nc.scalar.activation(
                dst[0:hs, :], ps[0:hs, :], AF.Identity, bias=bc_sb[0:hs, t:t + 1]
            )
            psy = pp_pre.tile([128, ISH], f32, tag="pre64", bufs=1)
            for k in range(KD):
                nc.tensor.matmul(
                    psy[0:hs, :],
                    lhsT=w1y_sb[:, k * H + 128 * t: k * H + 128 * t + hs],
                    rhs=yt_sb[:, k * ISH:(k + 1) * ISH],
                    start=(k == 0), stop=(k == KD - 1),
                )
            if t < 2:
                nc.vector.tensor_copy(hy_sb[0:hs, t * ISH:(t + 1) * ISH], psy[0:hs, :])
                nc.vector.tensor_copy(hyf_sb[0:hs, t * ISH:(t + 1) * ISH], psy[0:hs, :])
            else:
                nc.vector.tensor_copy(hyf_sb[0:hs, 2 * ISH:3 * ISH], psy[0:hs, :])
                # packed pair layout: col p <- (even col 2p at rows 0:44,
                # odd col 2p+1 at rows 64:108)
                evens = psy[0:hs, :].rearrange("p (a two) -> p two a", two=2)
                nc.vector.tensor_copy(hy2p[0:hs, :], evens[:, 0, :])
                nc.vector.tensor_copy(hy2p[64:64 + hs, :], evens[:, 1, :])
        nc.vector.tensor_copy(hxb2p[64:64 + HSZ[2], :], hxb2p[0:HSZ[2], :])

        # ---- main loop: 8 blocks of 8 rows, all z into ONE psum bank ----
        # row i -> strip c=(i//2)%4, psum row 32c + q, q = 2*(i//8) + i%2.
        def relu_op(eng, out_ap, in_ap, col_f32):
            if eng == 'A':
                nc.scalar.activation(out_ap, in_ap, AF.Relu, bias=col_f32)
            elif eng == 'P':
                nc.gpsimd.tensor_scalar(out_ap, in_ap, col_f32, 0.0, ALU.add, ALU.max)
            else:
                nc.vector.tensor_scalar(out_ap, in_ap, col_f32, 0.0, ALU.add, ALU.max)

        # w2q_sb blocks of 16 cols: sec0 = t0 (q=0..15), sec1 = t1,
        # sec2 = t2 pairs (m=0..7: w2t2 in col 2m rows 0:44 and col 2m+1
        # rows 64:108).
        def w2q_blk(sec, idx):
            off = 4 + (sec * 16 + idx) * 16
            return w2a_sb[:, off:off + 16]

        zbk = [
            pp_z.tile([128, N], f32, tag=f"zp{c}", name=f"zp{c}") for c in range(4)
        ]
        # ---- main loop: 8 blocks of 8 rows, block-major (strips
        # interleaved within each block); last block drains strip-major so
        # each strip's stop fires early for the tail exp.
        NB = ISH // 8
        for b in range(NB):
            last = b == NB - 1
            PATTERN20 = PATTERN20_6A if b < 6 else PATTERN20_5A
            if not last:
                iord = [8 * b + 2 * c + j for j in range(2) for c in range(4)]
                slots = [(i, 0) for i in iord] + [(i, 1) for i in iord] \
                    + [(c, 2) for c in range(4)]
            else:
                slots = []
                for c in range(4):
                    i0 = 8 * b + 2 * c
                    slots += [(i0, 0), (i0 + 1, 0), (i0, 1), (i0 + 1, 1), (c, 2)]
            rt = {}
            rps = {}
            for slot, (idx, t) in enumerate(slots):
                if t < 2:
                    r = rpool.tile([128, N], bf16, tag="r", bufs=32)
                    relu_op(
                        PATTERN20[slot], r[:], [hxb0, hxb1][t][:],
                        hyf_sb[:, t * ISH + idx: t * ISH + idx + 1],
                    )
                    rt[(idx, t)] = r
                else:
                    rp = rpool.tile([128, N], bf16, tag="rp", bufs=10)
                    relu_op(PATTERN20[slot], rp[:], hxb2p[:],
                            hy2p[:, 4 * b + idx:4 * b + idx + 1])
                    rps[idx] = rp

            def zmm(i, t):
                c = (i // 2) % 4
                q = 2 * (i // 8) + (i % 2)
                nc.tensor.matmul(
                    zbk[c][32 * c:32 * c + 16, :],
                    lhsT=w2q_blk(t, q), rhs=rt[(i, t)][:],
                    start=(b == 0 and t == 0 and i % 2 == 0), stop=False,
                    tile_position=(0, 32 * c),
                    skip_group_check=True,
                )

            def zpair(c):
                nc.tensor.matmul(
                    zbk[c][32 * c:32 * c + 16, :],
                    lhsT=w2q_blk(2, b), rhs=rps[c][:],
                    start=False, stop=last,
                    tile_position=(0, 32 * c),
                    skip_group_check=True,
                )

            if not last:
                for t in range(2):
                    for i in iord:
                        zmm(i, t)
                for c in range(4):
                    zpair(c)
            else:
                for c in range(4):
                    i0 = 8 * b + 2 * c
                    zmm(i0, 0)
                    zmm(i0 + 1, 0)
                    zmm(i0, 1)
                    zmm(i0 + 1, 1)
                    zpair(c)

            if b == 0:
                # ---- diag: dlog[i] = w2 . relu(hxd_i + hy_i + b1), fed
                # straight from PSUM (b1 added via a rank-1 K=1 matmul) ----
                dps = pp_d.tile([128, ISH], f32, tag="dps")
                for dt_ in range(HT):
                    hs = HSZ[dt_]
                    psd = pp_pre.tile([128, ISH], f32, tag="pre64", bufs=1)
                    for k in range(KD):
                        nc.tensor.matmul(
                            psd[0:hs, :],
                            lhsT=w1x_sb[:, k * H + 128 * dt_: k * H + 128 * dt_ + hs],
                            rhs=xtd_sb[:, k * ISH:(k + 1) * ISH],
                            start=(k == 0), stop=False,
                        )
                    nc.tensor.matmul(
                        psd[0:hs, :],
                        lhsT=b1r_sb[0:1, 128 * dt_:128 * dt_ + hs],
                        rhs=ones64[0:1, :],
                        start=False, stop=True,
                    )
                    dsum = rpool.tile([128, ISH], bf16, tag="dsum", bufs=2)
                    nc.vector.tensor_tensor(
                        dsum[0:hs, :], psd[0:hs, :],
                        hyf_sb[0:hs, dt_ * ISH:(dt_ + 1) * ISH], op=ALU.add,
                    )
                    dr = rpool.tile([128, ISH], bf16, tag="dr", bufs=2)
                    nc.vector.tensor_scalar(
                        dr[0:hs, :], dsum[0:hs, :], 0.0, None, ALU.max
                    )
                    nc.tensor.matmul(
                        dps[0:1, :],
                        lhsT=w2a_sb[0:hs, dt_ if dt_ < 2 else 2: (dt_ if dt_ < 2 else 2) + 1],
                        rhs=dr[0:hs, :],
                        start=(dt_ == 0), stop=(dt_ == HT - 1),
                    )
                nc.vector.tensor_copy(dlog[0:1, :], dps[0:1, :])
                nc.sync.dma_start(outD[0:1, :], dlog[0:1, :])

        # ---- tail: per-bank full exp + reduce ----
        # Full-partition ops only: partition-sliced Activation reads of PSUM
        # process just the first partition on HW. Junk rows (never written by
        # matmuls) land in unused sexp4 slots.
        for c in range(4):
            nc.scalar.activation(
                ee[:, c * N:(c + 1) * N], zbk[c][:], AF.Exp, bias=bc_sb[:, HT:HT + 1]
            )
            nc.vector.tensor_reduce(
                sexp4[:, c:c + 1], ee[:, c * N:(c + 1) * N],
                axis=mybir.AxisListType.X, op=ALU.add,
            )
            # issue each strip's output as soon as its reduce lands; spread
            # across both HWDGE queues so the issues overlap.
            eng = nc.sync if c % 2 == 0 else nc.scalar
            eng.dma_start(
                outS[16 * c:16 * (c + 1), :], sexp4[32 * c:32 * c + 16, c:c + 1]
            )

        for p in (pp_d, pp_z, pp_pre, rpool, cpool):
            p.release()

    nc.finalize()
    return nc


def _get_module():
    if "nc" not in _CACHE:
        _CACHE["nc"] = _build_module()
    return _CACHE["nc"]


def kernel(**inputs) -> np.ndarray:
    import ml_dtypes
    from concourse.bass_utils import run_bass_kernel_spmd

    bf = ml_dtypes.bfloat16
    x = np.ascontiguousarray(np.asarray(inputs["x_samples"], dtype=np.float32))
    y = np.ascontiguousarray(np.asarray(inputs["y_samples"], dtype=np.float32))
    W1 = np.asarray(inputs["W1"], dtype=np.float32)
    b1 = np.asarray(inputs["b1"], dtype=np.float32).reshape(H)
    W2 = np.asarray(inputs["W2"], dtype=np.float32)
    b2 = float(np.asarray(inputs["b2"], dtype=np.float32).reshape(1)[0])

    xT = np.ascontiguousarray(x.T.astype(bf))             # [768, 512]
    w1xT = np.ascontiguousarray(W1[:, :XD].T.astype(bf))  # [768, 300]
    w1yT = np.ascontiguousarray(W1[:, XD:].T.astype(bf))  # [768, 300]

    bcons = np.zeros((128, HT + 1), np.float32)
    w2 = W2.reshape(H)
    w2p = np.zeros((128, 4), bf)
    for t in range(2):
        bcons[:, t] = b1[128 * t:128 * (t + 1)]
        w2p[:, t] = w2[128 * t:128 * (t + 1)].astype(bf)
    hs2 = HSZ[2]
    bcons[:hs2, 2] = b1[256:256 + hs2]
    bcons[:, 3] = b2
    w2p[:hs2, 2] = w2[256:256 + hs2].astype(bf)
    w2p[64:64 + hs2, 3] = w2[256:256 + hs2].astype(bf)

    # w2q: sec0/sec1 = 16 blocks [128,16] with w2 t-chunk in col q; sec2 =
    # 8 pair blocks with w2 t2-chunk in col 2m rows 0:44 and col 2m+1 rows
    # 64:108.
    w2qm = np.zeros((128, 40 * 16), bf)
    for q in range(16):
        w2qm[:, (0 + q) * 16 + q] = w2[0:128].astype(bf)
        w2qm[:, (16 + q) * 16 + q] = w2[128:256].astype(bf)
    for m in range(8):
        w2qm[:hs2, (32 + m) * 16 + 2 * m] = w2[256:256 + hs2].astype(bf)
        w2qm[64:64 + hs2, (32 + m) * 16 + 2 * m + 1] = w2[256:256 + hs2].astype(bf)
    w2all = np.concatenate([w2p, w2qm], axis=1)
    b1row = np.zeros((1, 3 * 128), bf)
    b1row[0, 0:H] = 0.0  # layout below
    for t in range(2):
        b1row[0, 128 * t:128 * (t + 1)] = b1[128 * t:128 * (t + 1)].astype(bf)
    b1row[0, 256:256 + hs2] = b1[256:256 + hs2].astype(bf)

    in_maps = []
    for c in range(NCORES):
        sl = slice(c * ISH, (c + 1) * ISH)
        in_maps.append({
            "xT": xT,
            "w1xT": w1xT,
            "w1yT": w1yT,
            "yT": np.ascontiguousarray(y[sl].T.astype(bf)),   # [768, 64]
            "xTd": np.ascontiguousarray(x[sl].T.astype(bf)),  # [768, 64]
            "bcons": bcons,
            "w2all": w2all,
            "b1row": b1row,
        })

    nc = _get_module()
    res = run_bass_kernel_spmd(
        nc, in_maps, core_ids=list(range(NCORES)), trace=TRACE
    )
    global LAST_RESULTS
    LAST_RESULTS = res

    # device outS row 16c+q holds S for local i with c=(i//2)%4, q=2(i//8)+i%2
    ii = np.arange(ISH)
    perm = 16 * ((ii // 2) % 4) + 2 * (ii // 8) + (ii % 2)
    S = np.concatenate(
        [r["outS"].reshape(ISH)[perm] for r in res.results]
    ).astype(np.float64)
    d = np.concatenate([r["outD"].reshape(ISH) for r in res.results]).astype(np.float64)
    v = d + b2
    t0 = np.log1p(np.exp(-np.abs(v))) + np.maximum(v, 0.0)   # softplus(diag + b2)
    lse = np.log(float(N) + S)
    val = t0.mean() - (lse.mean() - math.log(N))
    return np.float32(val)



# revision 14
# speedup vs baseline: 1.1197x; 1.1197x over previous
"""InfoNCE lower-bound kernel for 8 Trainium2 NeuronCores (v2).

Math (reference):
  hx = x @ W1x.T ; hy = y @ W1y.T            [N, H]
  z_ij = relu(hx[j] + hy[i] + b1) . w2       (logit WITHOUT b2)
  T1[i,j] = softplus(z_ij + b2)
  T0[i]   = T1[i,i]
  lse[i]  = log(sum_j exp(T1[i,j])) = log(N + sum_j exp(z_ij + b2))
  out     = mean(T0) - (mean(lse) - log N)

Sharding: data-parallel over i (rows of the pair grid); each core gets 64
rows of y, x and params replicated.

v2 design decisions (vs the fp32 baseline):
  * bf16 operands everywhere on the grid path: matmuls run at 1 cycle/row
    (fp32 pays 4), DVE elementwise runs in packed mode, DMA bytes halve.
  * The relu grid pass is split across DVE and Activation via a tunable
    assignment table (Pool's tensor_scalar ucode is ~8us/op — unusable).
  * H=300 is tiled 128+128+44; the 44-tail relu work for two adjacent rows
    is packed into one [128,512] op (rows 0:44 and 64:108), cutting the
    elementwise op count from 192 to 160.
  * z matvecs use an M=16 stationary [K,16] holding w2 in column q_i
    (zeros elsewhere), at tile_position col strips (0,32c): row i lands at
    PSUM partition 32c+q of bank c (c=(i//2)%4, q=2(i//8)+i%2). Each strip
    accumulates into its OWN bank: concurrent col-strip matmuls accumulating
    into one bank corrupt each other on HW (verified in isolation). All
    matvecs are K=128 at row position 0 (a packed t2 pair is ONE matmul
    with w2 in col q0 rows 0:44 and col q0+1 rows 64:108); adding the zero
    columns is exact in fp32 PSUM.
  * exp then reads each bank directly with full-128-partition ops
    (partition-sliced Act reads of PSUM only process one partition on HW),
    followed by one DVE free-axis reduce per bank. No extraction copies.
  * Device ships per-row partial results (sum_j exp(z+b2) and diag logits);
    the final ln/softplus/means over 128 floats per core run on the host.
"""

import math

import numpy as np

N = 512
XD = 768
YD = 768
H = 300
NCORES = 8
ISH = N // NCORES   # 64 rows per core
KD = XD // 128      # 6 contraction tiles of 128
HT = 3              # h tiles: 128, 128, 44
HSZ = [128, 128, H - 256]

# Engine split per block of 8 rows: DVE (vector) takes 6 of 8 rows' t0/t1
# relu in bf16 (4x perf mode); Activation takes rows {2,5} (rel.) writing
# fp8 PAIR tiles [128, 1024] = [t0 | t1] consumed by ONE DoubleRow fp8
# matmul each (2 k-tiles per stream, 0.5 cyc/row), plus the t2 tail for
# strips 0 (all blocks) and 1 (blocks 0-5) as fp8 pair-of-blocks tiles.
# Act sheds t2 work in blocks 6-7 so it isn't the laggard into the exp tail.
# Pool (gpsimd) is useless here: tensor_scalar ucode ~8 us per [128,512] op.
ACT_I = (2, 5)                    # rel. rows on Act -> fp8 DoubleRow
IORD_DVE = (0, 3, 4, 6, 1, 7)     # strip-start rows (0,3,4,6) first
START_I = (0, 3, 4, 6)            # first matmul into each strip's bank

_CACHE = {}
TRACE = False
LAST_RESULTS = None


def _build_module():
    import concourse.bacc as bacc
    import concourse.mybir as mybir
    from concourse.tile import TileContext

    f32 = mybir.dt.float32
    bf16 = mybir.dt.bfloat16
    f8 = mybir.dt.float8e4
    AF = mybir.ActivationFunctionType
    ALU = mybir.AluOpType
    DR = mybir.MatmulPerfMode.DoubleRow

    nc = bacc.Bacc("TRN2", target_bir_lowering=False, debug=False)

    # Per-core inputs (SPMD: same shapes, different data for yT/xTd). All
    # pre-formatted on the host into the EXACT SBUF tile layout so every DMA
    # is a flat [128, F] contiguous copy (one 2-D descriptor — the v2 kernel's
    # rearranged 3-D DMAs cost ~688ns of descriptor generation per issue).
    xt_p = nc.dram_tensor("xt_p", [128, KD * N], bf16, kind="ExternalInput")
    w1x_p = nc.dram_tensor("w1x_p", [128, KD * H], bf16, kind="ExternalInput")
    w1y_p = nc.dram_tensor("w1y_p", [128, KD * H], bf16, kind="ExternalInput")
    yt_p = nc.dram_tensor("yt_p", [128, KD * ISH], bf16, kind="ExternalInput")
    xtd_p = nc.dram_tensor("xtd_p", [128, KD * ISH], bf16, kind="ExternalInput")
    bcons = nc.dram_tensor("bcons", [128, HT + 1], f32, kind="ExternalInput")  # b1 cols 0:3, b2 col 3
    w2all = nc.dram_tensor("w2all", [128, 4 + 40 * 16], bf16, kind="ExternalInput")  # diag w2 | w2q blocks
    w2f8 = nc.dram_tensor("w2f8", [128, 16 * 32 + 8 * 16], f8, kind="ExternalInput")  # DoubleRow weights
    b1row = nc.dram_tensor("b1row", [1, 3 * 128], bf16, kind="ExternalInput")  # b1 chunks as rows
    outS = nc.dram_tensor("outS", [128, 4], f32, kind="ExternalOutput")  # sexp4 image
    outD = nc.dram_tensor("outD", [1, ISH], f32, kind="ExternalOutput")  # diag logits

    with TileContext(nc) as tc:
        cpool = tc.alloc_tile_pool(name="consts", bufs=1)
        rpool = tc.alloc_tile_pool(name="work", bufs=32)
        pp_pre = tc.alloc_tile_pool(name="pp_pre", bufs=1, space="PSUM")
        pp_z = tc.alloc_tile_pool(name="pp_z", bufs=1, space="PSUM")
        pp_d = tc.alloc_tile_pool(name="pp_d", bufs=1, space="PSUM")

        # ---- constant tiles ----
        xt_sb = cpool.tile([128, KD * N], bf16, tag="xt")
        w1x_sb = cpool.tile([128, KD * H], bf16, tag="w1x")
        w1y_sb = cpool.tile([128, KD * H], bf16, tag="w1y")
        yt_sb = cpool.tile([128, KD * ISH], bf16, tag="yt")
        xtd_sb = cpool.tile([128, KD * ISH], bf16, tag="xtd")
        bc_sb = cpool.tile([128, HT + 1], f32, tag="bc")
        w2a_sb = cpool.tile([128, 4 + 40 * 16], bf16, tag="w2a")
        w2f8_sb = cpool.tile([128, 16 * 32 + 8 * 16], f8, tag="w2f8")

        hxb0 = cpool.tile([128, N], bf16, tag="hxb0")    # relu-arg x part (+b1), t0
        hxb1 = cpool.tile([128, N], bf16, tag="hxb1")    # t1
        hxb2p = cpool.tile([128, N], bf16, tag="hxb2p")  # t2 packed (rows 0:44, 64:108)
        hy_sb = cpool.tile([128, 2 * ISH], bf16, tag="hy")     # t0 | t1 columns
        hyf_sb = cpool.tile([128, HT * ISH], f32, tag="hyf")   # fp32 twin (scalar/bias/diag)
        hy2p = cpool.tile([128, ISH // 2], f32, tag="hy2p")    # packed t2 pairs
        b1r_sb = cpool.tile([1, 3 * 128], bf16, tag="b1r")
        ones64 = cpool.tile([1, ISH], bf16, tag="ones64")
        ee = cpool.tile([128, 4 * N], bf16, tag="ee")
        sexp4 = cpool.tile([128, 4], f32, tag="sexp4")
        dlog = cpool.tile([1, ISH], f32, tag="dlog")

        # ---- input DMAs: flat contiguous copies (host pre-formats into the
        # exact SBUF layout — one 2-D descriptor each, vs ~688ns of
        # descriptor-generation per rearranged 3-D DMA in v2), interleaved
        # across both HWDGE issue engines (sync + scalar), critical path
        # (hxb = xt+w1x) first. Halves let psum accumulation start early.
        def half(eng, dst, src, h):
            w = dst.shape[1] // 2
            eng.dma_start(dst[:, h * w:(h + 1) * w], src[:, h * w:(h + 1) * w])
        half(nc.sync, xt_sb, xt_p, 0)
        half(nc.scalar, w1x_sb, w1x_p, 0)
        half(nc.sync, xt_sb, xt_p, 1)
        half(nc.scalar, w1x_sb, w1x_p, 1)
        nc.scalar.dma_start(bc_sb[:], bcons[:])
        nc.sync.dma_start(w2f8_sb[:], w2f8[:])
        half(nc.sync, w1y_sb, w1y_p, 0)
        half(nc.scalar, w1y_sb, w1y_p, 1)
        nc.sync.dma_start(yt_sb[:], yt_p[:])
        nc.scalar.dma_start(w2a_sb[:], w2all[:])
        nc.sync.dma_start(xtd_sb[:], xtd_p[:])
        nc.scalar.dma_start(b1r_sb[:], b1row[:])

        nc.gpsimd.memset(ones64[:], 1.0)
        # zero the packed-t2 operand tiles before their writers fill the
        # live rows, so the pair matmul's zero-weight rows multiply finite
        # values (NaN * 0 = NaN).
        nc.gpsimd.memset(hxb2p[:], 0.0)
        nc.gpsimd.memset(hy2p[:], 0.0)

        # ---- preamble: hxb (x part, +b1) and hy interleaved per h-tile ----
        for t in range(HT):
            hs = HSZ[t]
            ps = pp_pre.tile([128, N], f32, tag="pre512", bufs=2)
            for k in range(KD):
                nc.tensor.matmul(
                    ps[0:hs, :],
                    lhsT=w1x_sb[:, k * H + 128 * t: k * H + 128 * t + hs],
                    rhs=xt_sb[:, k * N:(k + 1) * N],
                    start=(k == 0), stop=(k == KD - 1),
                )
            dst = [hxb0, hxb1, hxb2p][t]
            nc.scalar.activation(
                dst[0:hs, :], ps[0:hs, :], AF.Identity, bias=bc_sb[0:hs, t:t + 1]
            )
            psy = pp_pre.tile([128, ISH], f32, tag="pre64", bufs=1)
            for k in range(KD):
                nc.tensor.matmul(
                    psy[0:hs, :],
                    lhsT=w1y_sb[:, k * H + 128 * t: k * H + 128 * t + hs],
                    rhs=yt_sb[:, k * ISH:(k + 1) * ISH],
                    start=(k == 0), stop=(k == KD - 1),
                )
            if t < 2:
                nc.vector.tensor_copy(hy_sb[0:hs, t * ISH:(t + 1) * ISH], psy[0:hs, :])
                nc.vector.tensor_copy(hyf_sb[0:hs, t * ISH:(t + 1) * ISH], psy[0:hs, :])
            else:
                nc.vector.tensor_copy(hyf_sb[0:hs, 2 * ISH:3 * ISH], psy[0:hs, :])
                # packed pair layout: col p <- (even col 2p at rows 0:44,
                # odd col 2p+1 at rows 64:108)
                evens = psy[0:hs, :].rearrange("p (a two) -> p two a", two=2)
                nc.vector.tensor_copy(hy2p[0:hs, :], evens[:, 0, :])
                nc.vector.tensor_copy(hy2p[64:64 + hs, :], evens[:, 1, :])
        nc.vector.tensor_copy(hxb2p[64:64 + HSZ[2], :], hxb2p[0:HSZ[2], :])

        # ---- main loop: 8 blocks of 8 rows over TWO psum banks ----
        # Row i (local 0:64) -> bank i//32 (PE col strip 0 / 32, psum
        # partitions 0:32 / 32:64), row q = i%32. Every z matmul writes its
        # bank's FULL [32, 512] region (zero weight cols elsewhere), so all
        # accumulation into a bank is single-strip sequential (safe — the
        # cross-strip concurrent case corrupts on HW). Bank 0 finishes after
        # block 3, so its exp(+accum reduce) runs DURING blocks 4-7; only
        # bank 1's exp is in the tail.
        # Weight windows: w2 chunk placed at col 31 of a 63/64-wide strip;
        # lhsT = strip[31-q : 63-q] puts it at local col q -> psum row q.
        W_T0, W_T1, W_T2 = 4, 67, 130    # col offsets of strips in w2a_sb

        def relu_op(eng, out_ap, in_ap, col_f32):
            if eng == 'A':
                nc.scalar.activation(out_ap, in_ap, AF.Relu, bias=col_f32)
            else:
                nc.vector.tensor_scalar(out_ap, in_ap, col_f32, 0.0, ALU.add, ALU.max)

        zbk = [
            pp_z.tile([128, N], f32, tag=f"zp{c}", name=f"zp{c}") for c in range(2)
        ]
        escr = cpool.tile([128, 2 * N], bf16, tag="escr")  # exp out (unread)

        NB = ISH // 8
        rp2t = {}   # p -> fp8 pair-of-blocks t2 tile
        for b in range(NB):
            bank = b // 4
            q0 = 8 * (b % 4)           # q of the block's first row
            # --- relu ops ---
            rt = {}    # (i_rel, t) -> bf16 tile (DVE rows)
            rf8 = {}   # i_rel -> fp8 [128, 1024] pair tile (Act rows)
            rps = {}   # p -> bf16 t2 pair tile (DVE pairs 2, 3)
            for i_rel in IORD_DVE:
                for t in range(2):
                    r = rpool.tile([128, N], bf16, tag="r", bufs=28)
                    relu_op(
                        'D', r[:], [hxb0, hxb1][t][:],
                        hyf_sb[:, t * ISH + 8 * b + i_rel: t * ISH + 8 * b + i_rel + 1],
                    )
                    rt[(i_rel, t)] = r
            for i_rel in ACT_I:
                rf = rpool.tile([128, 2 * N], f8, tag="rf8", bufs=6)
                for t in range(2):
                    relu_op(
                        'A', rf[:, t * N:(t + 1) * N], [hxb0, hxb1][t][:],
                        hyf_sb[:, t * ISH + 8 * b + i_rel: t * ISH + 8 * b + i_rel + 1],
                    )
                rf8[i_rel] = rf
            for p in (0, 1):           # Act fp8 t2, paired across (b, b+1)
                if b % 2 == 0:
                    rp2t[p] = rpool.tile([128, 2 * N], f8, tag="rp2", bufs=4)
                relu_op('A', rp2t[p][:, (b % 2) * N:(b % 2 + 1) * N], hxb2p[:],
                        hy2p[:, 4 * b + p:4 * b + p + 1])
            for p in (2, 3):           # DVE bf16 t2 pairs
                rp = rpool.tile([128, N], bf16, tag="rp", bufs=6)
                relu_op('D', rp[:], hxb2p[:], hy2p[:, 4 * b + p:4 * b + p + 1])
                rps[p] = rp

            # --- z matmuls (PE order; single strip per bank) ---
            zo = zbk[bank][32 * bank:32 * bank + 32, :]
            tp = (0, 32 * bank)

            def zmm(i_rel, t, start=False):
                q = q0 + i_rel
                nc.tensor.matmul(
                    zo, lhsT=w2a_sb[:, [W_T0, W_T1][t] + 31 - q:
                                    [W_T0, W_T1][t] + 63 - q],
                    rhs=rt[(i_rel, t)][:], start=start, stop=False,
                    tile_position=tp, skip_group_check=True,
                )

            def zdr(i_rel):
                q = q0 + i_rel
                nc.tensor.matmul(
                    zo,
                    lhsT=w2f8_sb[:, 0:126]
                    .rearrange("p (two s) -> p two s", two=2)[:, :, 31 - q:63 - q],
                    rhs=rf8[i_rel][:].rearrange("p (two f) -> p two f", two=2),
                    start=False, stop=False, perf_mode=DROW,
                    tile_position=tp, skip_group_check=True,
                )

            def zpair(p):
                q = q0 + 2 * p
                nc.tensor.matmul(
                    zo, lhsT=w2a_sb[:, W_T2 + 31 - q:W_T2 + 63 - q],
                    rhs=rps[p][:], start=False, stop=False,
                    tile_position=tp, skip_group_check=True,
                )

            def zdr_t2(p, stop):
                q = 8 * (b % 4 - 1) + 2 * p    # q-pair of the EVEN block
                nc.tensor.matmul(
                    zo,
                    lhsT=w2f8_sb[:, 126:254]
                    .rearrange("p (two s) -> p two s", two=2)[:, :, 31 - q:63 - q],
                    rhs=rp2t[p][:].rearrange("p (two f) -> p two f", two=2),
                    start=False, stop=stop, perf_mode=DROW,
                    tile_position=tp, skip_group_check=True,
                )

            for t in range(2):
                for i_rel in IORD_DVE:
                    zmm(i_rel, t, start=(b % 4 == 0 and t == 0 and i_rel == IORD_DVE[0]))
            for i_rel in ACT_I:
                zdr(i_rel)
            zpair(2)
            zpair(3)
            if b % 2 == 1:
                zdr_t2(0, stop=False)
                zdr_t2(1, stop=(b % 4 == 3))

            if b % 4 == 3:
                # bank complete: exp(z + b2) with fused free-axis accumulate.
                # Valid rows are partitions 32*bank : 32*bank+32; the rest of
                # the accumulator column is junk the host ignores.
                nc.scalar.activation(
                    escr[:, bank * N:(bank + 1) * N], zbk[bank][:], AF.Exp,
                    bias=bc_sb[:, HT:HT + 1],
                    accum_out=sexp4[:, bank:bank + 1],
                )

            if b == 2:
                # ---- diag: dlog[i] = w2 . relu(hxd_i + hy_i + b1), fed
                # straight from PSUM (b1 added via a rank-1 K=1 matmul);
                # at b==2 so the xtd/w2all/b1row DMAs can land after the
                # grid has already started ----
                dps = pp_d.tile([128, ISH], f32, tag="dps")
                for dt_ in range(HT):
                    hs = HSZ[dt_]
                    psd = pp_pre.tile([128, ISH], f32, tag="pre64", bufs=1)
                    for k in range(KD):
                        nc.tensor.matmul(
                            psd[0:hs, :],
                            lhsT=w1x_sb[:, k * H + 128 * dt_: k * H + 128 * dt_ + hs],
                            rhs=xtd_sb[:, k * ISH:(k + 1) * ISH],
                            start=(k == 0), stop=False,
                        )
                    nc.tensor.matmul(
                        psd[0:hs, :],
                        lhsT=b1r_sb[0:1, 128 * dt_:128 * dt_ + hs],
                        rhs=ones64[0:1, :],
                        start=False, stop=True,
                    )
                    dsum = rpool.tile([128, ISH], bf16, tag="dsum", bufs=2)
                    nc.vector.tensor_tensor(
                        dsum[0:hs, :], psd[0:hs, :],
                        hyf_sb[0:hs, dt_ * ISH:(dt_ + 1) * ISH], op=ALU.add,
                    )
                    dr = rpool.tile([128, ISH], bf16, tag="dr", bufs=2)
                    nc.vector.tensor_scalar(
                        dr[0:hs, :], dsum[0:hs, :], 0.0, None, ALU.max
                    )
                    nc.tensor.matmul(
                        dps[0:1, :],
                        lhsT=w2a_sb[0:hs, dt_ if dt_ < 2 else 2: (dt_ if dt_ < 2 else 2) + 1],
                        rhs=dr[0:hs, :],
                        start=(dt_ == 0), stop=(dt_ == HT - 1),
                    )
                nc.vector.tensor_copy(dlog[0:1, :], dps[0:1, :])
                nc.sync.dma_start(outD[0:1, :], dlog[0:1, :])

        # ---- tail: per-bank full exp + reduce ----
        # Full-partition ops only: partition-sliced Activation reads of PSUM
        # process just the first partition on HW. Junk rows (never written by
        # matmuls) land in unused sexp4 slots.
        for c in range(4):
            nc.scalar.activation(
                ee[:, c * N:(c + 1) * N], zbk[c][:], AF.Exp, bias=bc_sb[:, HT:HT + 1]
            )
            nc.vector.tensor_reduce(
                sexp4[:, c:c + 1], ee[:, c * N:(c + 1) * N],
                axis=mybir.AxisListType.X, op=ALU.add,
            )
        # one output DMA for all 4 strips; host decodes row 32c+q of col c.
        nc.sync.dma_start(outS[:, :], sexp4[:, :])

        for p in (pp_d, pp_z, pp_pre, rpool, cpool):
            p.release()

    nc.finalize()
    return nc


def _get_module():
    if "nc" not in _CACHE:
        _CACHE["nc"] = _build_module()
    return _CACHE["nc"]


def kernel(**inputs) -> np.ndarray:
    import ml_dtypes
    from concourse.bass_utils import run_bass_kernel_spmd

    bf = ml_dtypes.bfloat16
    x = np.ascontiguousarray(np.asarray(inputs["x_samples"], dtype=np.float32))
    y = np.ascontiguousarray(np.asarray(inputs["y_samples"], dtype=np.float32))
    W1 = np.asarray(inputs["W1"], dtype=np.float32)
    b1 = np.asarray(inputs["b1"], dtype=np.float32).reshape(H)
    W2 = np.asarray(inputs["W2"], dtype=np.float32)
    b2 = float(np.asarray(inputs["b2"], dtype=np.float32).reshape(1)[0])

    def sbuf_fmt(aT):
        # [KD*128, F] transposed matrix -> SBUF tile layout [128, KD*F]
        # (row p, col k*F+f = aT[128k+p, f]), contiguous.
        kd = aT.shape[0] // 128
        return np.ascontiguousarray(
            aT.reshape(kd, 128, -1).transpose(1, 0, 2).reshape(128, -1).astype(bf)
        )

    xt_p = sbuf_fmt(x.T)             # [128, 6*512]
    w1x_p = sbuf_fmt(W1[:, :XD].T)   # [128, 6*300]
    w1y_p = sbuf_fmt(W1[:, XD:].T)   # [128, 6*300]

    bcons = np.zeros((128, HT + 1), np.float32)
    w2 = W2.reshape(H)
    w2p = np.zeros((128, 4), bf)
    for t in range(2):
        bcons[:, t] = b1[128 * t:128 * (t + 1)]
        w2p[:, t] = w2[128 * t:128 * (t + 1)].astype(bf)
    hs2 = HSZ[2]
    bcons[:hs2, 2] = b1[256:256 + hs2]
    bcons[:, 3] = b2
    w2p[:hs2, 2] = w2[256:256 + hs2].astype(bf)
    w2p[64:64 + hs2, 3] = w2[256:256 + hs2].astype(bf)

    # w2q: sec0/sec1 = 16 blocks [128,16] with w2 t-chunk in col q; sec2 =
    # 8 pair blocks with w2 t2-chunk in col 2m rows 0:44 and col 2m+1 rows
    # 64:108.
    w2qm = np.zeros((128, 40 * 16), bf)
    for q in range(16):
        w2qm[:, (0 + q) * 16 + q] = w2[0:128].astype(bf)
        w2qm[:, (16 + q) * 16 + q] = w2[128:256].astype(bf)
    for m in range(8):
        w2qm[:hs2, (32 + m) * 16 + 2 * m] = w2[256:256 + hs2].astype(bf)
        w2qm[64:64 + hs2, (32 + m) * 16 + 2 * m + 1] = w2[256:256 + hs2].astype(bf)
    w2all = np.concatenate([w2p, w2qm], axis=1)
    b1row = np.zeros((1, 3 * 128), bf)
    b1row[0, 0:H] = 0.0  # layout below
    for t in range(2):
        b1row[0, 128 * t:128 * (t + 1)] = b1[128 * t:128 * (t + 1)].astype(bf)
    b1row[0, 256:256 + hs2] = b1[256:256 + hs2].astype(bf)

    in_maps = []
    for c in range(NCORES):
        sl = slice(c * ISH, (c + 1) * ISH)
        in_maps.append({
            "xt_p": xt_p,
            "w1x_p": w1x_p,
            "w1y_p": w1y_p,
            "yt_p": sbuf_fmt(y[sl].T),   # [128, 6*64]
            "xtd_p": sbuf_fmt(x[sl].T),  # [128, 6*64]
            "bcons": bcons,
            "w2all": w2all,
            "b1row": b1row,
        })

    nc = _get_module()
    res = run_bass_kernel_spmd(
        nc, in_maps, core_ids=list(range(NCORES)), trace=TRACE
    )
    global LAST_RESULTS
    LAST_RESULTS = res

    # device outS is the [128, 4] sexp4 image: S for local i sits at
    # row 32c+q, col c with c=(i//2)%4, q=2(i//8)+i%2
    ii = np.arange(ISH)
    cc = (ii // 2) % 4
    qq = 2 * (ii // 8) + (ii % 2)
    S = np.concatenate(
        [r["outS"].reshape(128, 4)[32 * cc + qq, cc] for r in res.results]
    ).astype(np.float64)
    d = np.concatenate([r["outD"].reshape(ISH) for r in res.results]).astype(np.float64)
    v = d + b2
    t0 = np.log1p(np.exp(-np.abs(v))) + np.maximum(v, 0.0)   # softplus(diag + b2)
    lse = np.log(float(N) + S)
    val = t0.mean() - (lse.mean() - math.log(N))
    return np.float32(val)

